# revision 29
# baseline (speedup 1.0000x reference)
"""Trainium2 Bass kernel for nn_ClusterLoss (topk_masking).

Strategy (8 NeuronCores, data-parallel over the 4096 selected rows):
  - Host shards mc_rows and the gathered row_scores rows across cores
    (512 rows/core). Each score is negated and quantized to a 16-bit
    word that is a valid positive bf16: the 8-bit exponent field holds
    the quantized relative value q (resolution ln2/64 so exp(v) ==
    2^(q/64)) and the 7-bit mantissa holds the 128-column block id.
    bf16 ordering == (value, block) lexicographic ordering, so MAX8 /
    max-reduce rank correctly and half the HBM bytes move vs f32.
  - Device, per 128-row tile: chunked MAX8 over the 10000 packed words
    -> top-3 (value, block) words. The 3 winner blocks are re-fetched
    with indirect DMA (3 x 128 words/row) and MAX8 + max_index recover
    the exact in-block position -> global column index. H rows (bf16)
    are gathered by that index; diff + Square-accumulate give the
    neighbor norms. Softmax weights come from the packed words
    directly: w^(1/64) (six chained Sqrts) == exp(v) modulo a <=1.1%
    mantissa factor that cancels in the normalization.
  - Masked-MSE residual and squared-norm partials run on a bf16 copy
    of a 1250-row slice of X/H/C/M, interleaved with the score stream.
  - Each core returns [128, 8] per-partition partial sums; host
    reduces and assembles the scalar loss.
"""

import sys

sys.path.insert(0, "/opt/trn_rl_repo")

import math

import ml_dtypes
import numpy as np

from concourse import bacc, bass, mybir, tile
from concourse.bass_utils import run_bass_kernel_spmd
from concourse.tile_rust import add_dep_helper

N, D, R = 10000, 256, 4096
NCORES = 8
RPC = R // NCORES          # score rows per core = 512
SLC = N // NCORES          # mse rows per core = 1250
P = 128
NT = RPC // P              # score row-tiles per core = 4
MSE_FD = SLC * D // P      # 2500
F32 = mybir.dt.float32
U32 = mybir.dt.uint32
U16 = mybir.dt.uint16
BF16 = mybir.dt.bfloat16

W = 128                    # columns per block (mantissa-encoded id)
NBLK = 80                  # blocks per padded row
NCOLP = NBLK * W           # padded row length = 10240
CHUNKS = [2560, 2560, 2560, 2320]   # covers the 10000 real columns

# bulk tensor column offsets: xs | hs | cs | ms | hsel | pbase bits
BO_X, BO_H, BO_C, BO_M = 0, MSE_FD, 2 * MSE_FD, 3 * MSE_FD
BO_HS = 4 * MSE_FD                  # 10000
BO_PB = BO_HS + NT * D              # 11024
BULK_COLS = BO_PB + NT * 2          # 11032 bf16 columns

QS = 64.0 / math.log(2.0)  # quant scale: exp(v) == 2**(q/64)
QR = 254.0 / QS            # representable value range below row max

_compiled = None


def _build_program():
    nc = bacc.Bacc("TRN2", target_bir_lowering=False, debug=False)

    scores = nc.dram_tensor("scores", [RPC, NCOLP], BF16, kind="ExternalInput").ap()
    hfull = nc.dram_tensor("hfull", [N, D], BF16, kind="ExternalInput").ap()
    bulk = nc.dram_tensor("bulk", [P, BULK_COLS], BF16, kind="ExternalInput").ap()
    out = nc.dram_tensor("out", [P, 8], F32, kind="ExternalOutput").ap()

    gview = scores.rearrange("r (b w) -> (r b) w", w=W)   # [RPC*NBLK, W]

    with tile.TileContext(nc) as tc:
        with (
            tc.tile_pool(name="sc", bufs=1) as sc_pool,
            tc.tile_pool(name="small", bufs=NT) as small,
            tc.tile_pool(name="hp", bufs=NT) as hpool,
            tc.tile_pool(name="acc", bufs=1) as acc,
            tc.tile_pool(name="mse", bufs=1) as msep,
        ):
            res_t = acc.tile([P, 8], F32, tag="res")
            nc.vector.memset(res_t[:], 0.0)
            nrm2all = acc.tile([P, NT * 3], F32, tag="n2all")
            uall = acc.tile([P, NT * 3], F32, tag="uall")

            # one bulk tile: xs | hs | cs | ms | hsel | pbase-bits
            bulkt = msep.tile([P, BULK_COLS], BF16, tag="bulkt")
            xt = bulkt[:, BO_X:BO_X + MSE_FD]
            ht = bulkt[:, BO_H:BO_H + MSE_FD]
            ct = bulkt[:, BO_C:BO_C + MSE_FD]
            mt = bulkt[:, BO_M:BO_M + MSE_FD]
            hst = bulkt[:, BO_HS:BO_HS + NT * D]
            baset = bulkt[:, BO_PB:BO_PB + NT * 2].bitcast(U32)   # [P, NT]

            # ---- DMA issue order (descriptor-count, not bytes, paces the
            # DGE): 2 full-width contiguous-row chunks per tile + 1 bulk.
            sc_tiles = [
                sc_pool.tile([P, NCOLP], BF16, tag=f"sc{t}", name=f"sct{t}")
                for t in range(NT)
            ]

            def queue_tile_dma(t, sizes):
                col = 0
                for w in sizes:
                    nc.sync.dma_start(
                        out=sc_tiles[t][:, col:col + w],
                        in_=scores[t * P:(t + 1) * P, col:col + w])
                    col += w

            # hsel+pbase early (needed by rescan/diff of tile 0); the big
            # mse block last — its consumers run in the tail when the DVE
            # is idle, so it must never delay the score stream.
            queue_tile_dma(0, CHUNKS)
            nc.sync.dma_start(out=bulkt[:, BO_HS:], in_=bulk[:, BO_HS:])
            queue_tile_dma(1, CHUNKS)
            queue_tile_dma(2, [5120, 4880])
            queue_tile_dma(3, [5120, 4880])
            nc.sync.dma_start(out=bulkt[:, 0:BO_HS], in_=bulk[:, 0:BO_HS])

            # ---- per-tile state
            m8s, bids, blk3s, top8s, poss, cols, hns = (
                [None] * NT, [None] * NT, [None] * NT, [None] * NT,
                [None] * NT, [None] * NT, [None] * NT)

            def find_stage(t):
                """chunk MAX8s + merge + block-id unpack (DVE), then
                GpSimd computes gather rows and fetches winner blocks."""
                m8h = small.tile([P, len(CHUNKS) * 8], BF16, tag="m8h")
                col = 0
                for h, w in enumerate(CHUNKS):
                    nc.vector.max(out=m8h[:, h * 8:(h + 1) * 8],
                                  in_=sc_tiles[t][:, col:col + w])
                    col += w
                m8 = small.tile([P, 8], BF16, tag="m8")
                nc.vector.max(out=m8[:], in_=m8h[:])
                bid16 = small.tile([P, 3], U16, tag="bid16")
                nc.vector.tensor_scalar(
                    out=bid16[:], in0=m8[:, 0:3].bitcast(U16), scalar1=W - 1,
                    scalar2=None, op0=mybir.AluOpType.bitwise_and)
                # bitVec ops can't change dtype; widen u16->u32 via add 0
                bid = small.tile([P, 3], U32, tag="bid")
                nc.vector.tensor_scalar(
                    out=bid[:], in0=bid16[:], scalar1=0,
                    scalar2=None, op0=mybir.AluOpType.add)
                rows = small.tile([P, 3], U32, tag="rows")
                nc.vector.tensor_tensor(
                    out=rows[:], in0=bid[:],
                    in1=baset[:, t:t + 1].to_broadcast([P, 3]),
                    op=mybir.AluOpType.add)
                blk3 = small.tile([P, 3 * W], BF16, tag="blk3")
                for k in range(3):
                    nc.gpsimd.indirect_dma_start(
                        out=blk3[:, k * W:(k + 1) * W], out_offset=None,
                        in_=gview,
                        in_offset=bass.IndirectOffsetOnAxis(
                            ap=rows[:, k:k + 1], axis=0))
                m8s[t], bids[t], blk3s[t] = m8, bid, blk3

            def rescan_stage(t):
                """Per-winner max_index in its own gathered block: the
                match position IS the in-block column offset, so
                col = bid*128 + pos with no segment-select math."""
                pos_all = small.tile([P, 24], U32, tag="pos_all")
                for k in range(3):
                    nc.vector.max_index(
                        out=pos_all[:, k * 8:(k + 1) * 8],
                        in_max=m8s[t][:, k:k + 1].to_broadcast([P, 8]),
                        in_values=blk3s[t][:, k * W:(k + 1) * W])
                colw = small.tile([P, 3], U32, tag="colw")
                nc.vector.tensor_scalar(
                    out=colw[:], in0=bids[t][:], scalar1=W, scalar2=None,
                    op0=mybir.AluOpType.mult)
                col3 = small.tile([P, 3], U32, tag="col3")
                nc.vector.tensor_tensor(
                    out=col3[:], in0=colw[:],
                    in1=pos_all[:].rearrange("p (k e) -> p k e", e=8)[:, :, 0],
                    op=mybir.AluOpType.add)
                hn = hpool.tile([P, 3 * D], BF16, tag="hn")
                for k in range(3):
                    nc.gpsimd.indirect_dma_start(
                        out=hn[:, k * D:(k + 1) * D], out_offset=None,
                        in_=hfull,
                        in_offset=bass.IndirectOffsetOnAxis(
                            ap=col3[:, k:k + 1], axis=0))
                hns[t] = hn
                # weight numerators: u = word^(1/64) == exp(v) * (1 +- 1.1%)
                nc.scalar.copy(out=uall[:, t * 3:(t + 1) * 3], in_=m8s[t][:, 0:3])

            def diff_stage(t, eng=None):
                """neighbor diffs (GpSimd bf16; DVE for the tail tile) +
                Square-accumulate (ACT)."""
                dif = hpool.tile([P, 3 * D], BF16, tag="dif")
                hb = hst[:, t * D:(t + 1) * D].unsqueeze(1).to_broadcast([P, 3, D])
                (eng or nc.gpsimd).tensor_tensor(
                    out=dif[:].rearrange("p (k d) -> p k d", k=3),
                    in0=hb, in1=hns[t][:].rearrange("p (k d) -> p k d", k=3),
                    op=mybir.AluOpType.subtract)
                for k in range(3):
                    nc.scalar.activation(
                        out=dif[:, k * D:(k + 1) * D],
                        in_=dif[:, k * D:(k + 1) * D],
                        func=mybir.ActivationFunctionType.Square,
                        accum_out=nrm2all[:, t * 3 + k:t * 3 + k + 1])

            # ---- interleaved schedule: keep the DVE fed with MAX8s while
            # gather chains for earlier tiles run on GpSimd/DMA. The bulk
            # tensor lands early, so the whole mse chain runs mid-kernel.
            sq = msep.tile([P, MSE_FD], BF16, tag="sq")
            find_stage(0)
            find_stage(1)
            rescan_stage(0)
            nc.scalar.activation(out=sq[:], in_=ht[:],
                                 func=mybir.ActivationFunctionType.Square,
                                 accum_out=res_t[:, 2:3])
            find_stage(2)
            rescan_stage(1)
            diff_stage(0)
            # mse residual chain: resid = ((x - h) + c) * m, in place
            nc.vector.tensor_tensor(out=xt[:], in0=xt[:], in1=ht[:],
                                    op=mybir.AluOpType.subtract)
            nc.scalar.activation(out=sq[:], in_=ct[:],
                                 func=mybir.ActivationFunctionType.Square,
                                 accum_out=res_t[:, 3:4])
            find_stage(3)
            rescan_stage(2)
            diff_stage(1)
            nc.vector.tensor_tensor(out=xt[:], in0=xt[:], in1=ct[:],
                                    op=mybir.AluOpType.add)
            nc.vector.tensor_tensor(out=xt[:], in0=xt[:], in1=mt[:],
                                    op=mybir.AluOpType.mult)
            rescan_stage(3)
            diff_stage(2)
            nc.scalar.activation(out=sq[:], in_=xt[:],
                                 func=mybir.ActivationFunctionType.Square,
                                 accum_out=res_t[:, 1:2])
            diff_stage(3, eng=nc.vector)

            # ---- phase B: softmax weights via six chained sqrts, then
            # weighted neighbor-norm dot, all on [P, 12].
            for _ in range(6):
                nc.scalar.sqrt(out=uall[:], in_=uall[:])
            nrmall = acc.tile([P, NT * 3], F32, tag="nrmall")
            nc.scalar.sqrt(out=nrmall[:], in_=nrm2all[:])
            s1 = acc.tile([P, NT], F32, tag="s1")
            nc.vector.tensor_reduce(
                out=s1[:], in_=uall[:].rearrange("p (t k) -> p t k", k=3),
                axis=mybir.AxisListType.X, op=mybir.AluOpType.add)
            r1 = acc.tile([P, NT], F32, tag="r1")
            nc.vector.reciprocal(out=r1[:], in_=s1[:])
            en = acc.tile([P, NT * 3], F32, tag="en")
            nc.vector.tensor_tensor(out=en[:], in0=uall[:], in1=nrmall[:],
                                    op=mybir.AluOpType.mult)
            dot = acc.tile([P, NT], F32, tag="dot")
            nc.vector.tensor_reduce(
                out=dot[:], in_=en[:].rearrange("p (t k) -> p t k", k=3),
                axis=mybir.AxisListType.X, op=mybir.AluOpType.add)
            simc = acc.tile([P, NT], F32, tag="simc")
            nc.vector.tensor_tensor(out=simc[:], in0=dot[:], in1=r1[:],
                                    op=mybir.AluOpType.mult)
            nc.vector.tensor_reduce(
                out=res_t[:, 0:1], in_=simc[:], axis=mybir.AxisListType.X,
                op=mybir.AluOpType.add)

            nc.sync.dma_start(out=out, in_=res_t[:])

    nc.compile()
    return nc


def _get_program():
    global _compiled
    if _compiled is None:
        _compiled = _build_program()
    return _compiled


def _pack_scores(row_scores, mc):
    """Negate + gather score rows, quantize the value into the bf16
    exponent field and the 128-column block id into the mantissa."""
    neg = -row_scores[mc]                                   # [R, N] f32
    m = neg.max(axis=1, keepdims=True)
    q = np.clip(np.rint((neg - m + QR) * QS), 0.0, 254.0).astype(np.uint16)
    blk = (np.arange(NCOLP, dtype=np.uint16) >> 7)          # [NCOLP]
    words = np.empty((R, NCOLP), dtype=np.uint16)
    words[:, :N] = (q << 7) | blk[:N]
    words[:, N:] = blk[N:]                                  # pad: q=0 losers
    return words.view(ml_dtypes.bfloat16)


def _make_in_maps(X, H, C, M, row_scores, mc_rows):
    mc = np.asarray(mc_rows).astype(np.int64)
    scores_p = _pack_scores(np.ascontiguousarray(row_scores), mc)
    bf = ml_dtypes.bfloat16
    Hb = H.astype(bf)
    hsel_g = Hb[mc]                                         # [R, D]
    pbase = (np.arange(P, dtype=np.uint32)[:, None] * NBLK
             + np.arange(NT, dtype=np.uint32)[None, :] * (P * NBLK))
    pbase_bits = pbase.view(np.uint16).view(bf)             # [P, NT*2]
    in_maps = []
    for c in range(NCORES):
        sl = slice(c * RPC, (c + 1) * RPC)
        rs = slice(c * SLC, (c + 1) * SLC)
        bulk = np.empty((P, BULK_COLS), dtype=bf)
        bulk[:, BO_X:BO_X + MSE_FD] = X[rs].astype(bf).reshape(P, MSE_FD)
        bulk[:, BO_H:BO_H + MSE_FD] = H[rs].astype(bf).reshape(P, MSE_FD)
        bulk[:, BO_C:BO_C + MSE_FD] = C[rs].astype(bf).reshape(P, MSE_FD)
        bulk[:, BO_M:BO_M + MSE_FD] = M[rs].astype(bf).reshape(P, MSE_FD)
        bulk[:, BO_HS:BO_HS + NT * D] = (
            hsel_g[sl].reshape(NT, P, D).transpose(1, 0, 2).reshape(P, NT * D))
        bulk[:, BO_PB:] = pbase_bits
        in_maps.append({
            "scores": np.ascontiguousarray(scores_p[sl]),
            "hfull": np.ascontiguousarray(Hb),
            "bulk": bulk,
        })
    return in_maps


def _finish(results):
    parts = np.stack([r["out"] for r in results]).astype(np.float64)  # [8,128,8]
    tot = parts.sum(axis=(0, 1))
    loss = tot[1] + tot[0] + 0.1 * np.sqrt(tot[3]) + 0.01 * np.sqrt(tot[2])
    return np.array(loss, dtype=np.float32)


def kernel(X, H, C, M, T, nM, row_scores, mc_rows, **_unused):
    X = np.asarray(X, dtype=np.float32)
    H = np.asarray(H, dtype=np.float32)
    C = np.asarray(C, dtype=np.float32)
    M = np.asarray(M, dtype=np.float32)
    row_scores = np.asarray(row_scores, dtype=np.float32)
    nc = _get_program()
    in_maps = _make_in_maps(X, H, C, M, row_scores, mc_rows)
    res = run_bass_kernel_spmd(nc, in_maps, list(range(NCORES)))
    return _finish(res.results)


def run_traced(X, H, C, M, T, nM, row_scores, mc_rows, **_unused):
    """Like kernel() but returns (loss, BassKernelResults) with trace."""
    nc = _get_program()
    in_maps = _make_in_maps(
        np.asarray(X, dtype=np.float32), np.asarray(H, dtype=np.float32),
        np.asarray(C, dtype=np.float32), np.asarray(M, dtype=np.float32),
        np.asarray(row_scores, dtype=np.float32), mc_rows)
    try:
        res = run_bass_kernel_spmd(nc, in_maps, list(range(NCORES)), trace=True)
    except ModuleNotFoundError:
        res = run_bass_kernel_spmd(nc, in_maps, list(range(NCORES)))
    return _finish(res.results), res


# revision 30
# speedup vs baseline: 1.0090x; 1.0090x over previous
"""Trainium2 Bass kernel for nn_ClusterLoss (topk_masking).

Strategy (8 NeuronCores, data-parallel over the 4096 selected rows):
  - Host shards mc_rows and the gathered row_scores rows across cores
    (512 rows/core). Each score is negated and quantized to a 16-bit
    word that is a valid positive bf16: the 8-bit exponent field holds
    the quantized relative value q (resolution ln2/64 so exp(v) ==
    2^(q/64)) and the 7-bit mantissa holds the 128-column block id.
    bf16 ordering == (value, block) lexicographic ordering, so MAX8 /
    max-reduce rank correctly and half the HBM bytes move vs f32.
  - Device, per 128-row tile: chunked MAX8 over the 10000 packed words
    -> top-3 (value, block) words. The 3 winner blocks are re-fetched
    with indirect DMA (3 x 128 words/row) and MAX8 + max_index recover
    the exact in-block position -> global column index. H rows (bf16)
    are gathered by that index; diff + Square-accumulate give the
    neighbor norms. Softmax weights come from the packed words
    directly: w^(1/64) (six chained Sqrts) == exp(v) modulo a <=1.1%
    mantissa factor that cancels in the normalization.
  - Masked-MSE residual and squared-norm partials run on a bf16 copy
    of a 1250-row slice of X/H/C/M, interleaved with the score stream.
  - Each core returns [128, 8] per-partition partial sums; host
    reduces and assembles the scalar loss.
"""

import sys

sys.path.insert(0, "/opt/trn_rl_repo")

import math

import ml_dtypes
import numpy as np

from concourse import bacc, bass, mybir, tile
from concourse.bass_utils import run_bass_kernel_spmd
from concourse.tile_rust import add_dep_helper

N, D, R = 10000, 256, 4096
NCORES = 8
RPC = R // NCORES          # score rows per core = 512
SLC = N // NCORES          # mse rows per core = 1250
P = 128
NT = RPC // P              # score row-tiles per core = 4
MSE_FD = SLC * D // P      # 2500
F32 = mybir.dt.float32
U32 = mybir.dt.uint32
U16 = mybir.dt.uint16
BF16 = mybir.dt.bfloat16

W = 128                    # columns per block (mantissa-encoded id)
NBLK = 80                  # blocks per padded row
NCOLP = NBLK * W           # padded row length = 10240
CHUNKS = [2560, 2560, 2560, 2320]   # covers the 10000 real columns

# bulk tensor column offsets: xs | hs | cs | ms | hsel | pbase bits
BO_X, BO_H, BO_C, BO_M = 0, MSE_FD, 2 * MSE_FD, 3 * MSE_FD
BO_HS = 4 * MSE_FD                  # 10000
BO_PB = BO_HS + NT * D              # 11024
BULK_COLS = BO_PB + NT * 2          # 11032 bf16 columns

QS = 64.0 / math.log(2.0)  # quant scale: exp(v) == 2**(q/64)
QR = 254.0 / QS            # representable value range below row max

_compiled = None


def _build_program():
    nc = bacc.Bacc("TRN2", target_bir_lowering=False, debug=False)

    scores = nc.dram_tensor("scores", [RPC, NCOLP], BF16, kind="ExternalInput").ap()
    hfull = nc.dram_tensor("hfull", [N, D], BF16, kind="ExternalInput").ap()
    bulk = nc.dram_tensor("bulk", [P, BULK_COLS], BF16, kind="ExternalInput").ap()
    out = nc.dram_tensor("out", [P, 8], F32, kind="ExternalOutput").ap()

    gview = scores.rearrange("r (b w) -> (r b) w", w=W)   # [RPC*NBLK, W]

    with tile.TileContext(nc) as tc:
        with (
            tc.tile_pool(name="sc", bufs=1) as sc_pool,
            tc.tile_pool(name="small", bufs=NT) as small,
            tc.tile_pool(name="hp", bufs=NT) as hpool,
            tc.tile_pool(name="acc", bufs=1) as acc,
            tc.tile_pool(name="mse", bufs=1) as msep,
        ):
            res_t = acc.tile([P, 8], F32, tag="res")
            nc.vector.memset(res_t[:], 0.0)
            nrm2all = acc.tile([P, NT * 3], F32, tag="n2all")
            uall = acc.tile([P, NT * 3], F32, tag="uall")

            # one bulk tile: xs | hs | cs | ms | hsel | pbase-bits
            bulkt = msep.tile([P, BULK_COLS], BF16, tag="bulkt")
            xt = bulkt[:, BO_X:BO_X + MSE_FD]
            ht = bulkt[:, BO_H:BO_H + MSE_FD]
            ct = bulkt[:, BO_C:BO_C + MSE_FD]
            mt = bulkt[:, BO_M:BO_M + MSE_FD]
            hst = bulkt[:, BO_HS:BO_HS + NT * D]
            baset = bulkt[:, BO_PB:BO_PB + NT * 2].bitcast(U32)   # [P, NT]

            # ---- DMA issue order (descriptor-count, not bytes, paces the
            # DGE): 2 full-width contiguous-row chunks per tile + 1 bulk.
            sc_tiles = [
                sc_pool.tile([P, NCOLP], BF16, tag=f"sc{t}", name=f"sct{t}")
                for t in range(NT)
            ]

            def queue_tile_dma(t, sizes):
                col = 0
                for w in sizes:
                    nc.sync.dma_start(
                        out=sc_tiles[t][:, col:col + w],
                        in_=scores[t * P:(t + 1) * P, col:col + w])
                    col += w

            # hsel+pbase early (needed by rescan/diff of tile 0); the big
            # mse block last — its consumers run in the tail when the DVE
            # is idle, so it must never delay the score stream.
            queue_tile_dma(0, CHUNKS)
            nc.sync.dma_start(out=bulkt[:, BO_HS:], in_=bulk[:, BO_HS:])
            queue_tile_dma(1, CHUNKS)
            queue_tile_dma(2, CHUNKS)
            queue_tile_dma(3, CHUNKS)
            nc.sync.dma_start(out=bulkt[:, 0:BO_HS], in_=bulk[:, 0:BO_HS])

            # ---- per-tile state
            m8s, bids, blk3s, top8s, poss, cols, hns = (
                [None] * NT, [None] * NT, [None] * NT, [None] * NT,
                [None] * NT, [None] * NT, [None] * NT)

            def find_stage(t):
                """chunk MAX8s + merge + block-id unpack (DVE), then
                GpSimd computes gather rows and fetches winner blocks."""
                m8h = small.tile([P, len(CHUNKS) * 8], BF16, tag="m8h")
                col = 0
                for h, w in enumerate(CHUNKS):
                    nc.vector.max(out=m8h[:, h * 8:(h + 1) * 8],
                                  in_=sc_tiles[t][:, col:col + w])
                    col += w
                m8 = small.tile([P, 8], BF16, tag="m8")
                nc.vector.max(out=m8[:], in_=m8h[:])
                bid16 = small.tile([P, 3], U16, tag="bid16")
                nc.vector.tensor_scalar(
                    out=bid16[:], in0=m8[:, 0:3].bitcast(U16), scalar1=W - 1,
                    scalar2=None, op0=mybir.AluOpType.bitwise_and)
                # bitVec ops can't change dtype; widen u16->u32 via add 0
                bid = small.tile([P, 3], U32, tag="bid")
                nc.vector.tensor_scalar(
                    out=bid[:], in0=bid16[:], scalar1=0,
                    scalar2=None, op0=mybir.AluOpType.add)
                rows = small.tile([P, 3], U32, tag="rows")
                nc.vector.tensor_tensor(
                    out=rows[:], in0=bid[:],
                    in1=baset[:, t:t + 1].to_broadcast([P, 3]),
                    op=mybir.AluOpType.add)
                blk3 = small.tile([P, 3 * W], BF16, tag="blk3")
                for k in range(3):
                    nc.gpsimd.indirect_dma_start(
                        out=blk3[:, k * W:(k + 1) * W], out_offset=None,
                        in_=gview,
                        in_offset=bass.IndirectOffsetOnAxis(
                            ap=rows[:, k:k + 1], axis=0))
                m8s[t], bids[t], blk3s[t] = m8, bid, blk3

            def rescan_stage(t):
                """Per-winner max_index in its own gathered block: the
                match position IS the in-block column offset, so
                col = bid*128 + pos with no segment-select math."""
                pos_all = small.tile([P, 24], U32, tag="pos_all")
                for k in range(3):
                    nc.vector.max_index(
                        out=pos_all[:, k * 8:(k + 1) * 8],
                        in_max=m8s[t][:, k:k + 1].to_broadcast([P, 8]),
                        in_values=blk3s[t][:, k * W:(k + 1) * W])
                colw = small.tile([P, 3], U32, tag="colw")
                nc.vector.tensor_scalar(
                    out=colw[:], in0=bids[t][:], scalar1=W, scalar2=None,
                    op0=mybir.AluOpType.mult)
                col3 = small.tile([P, 3], U32, tag="col3")
                nc.vector.tensor_tensor(
                    out=col3[:], in0=colw[:],
                    in1=pos_all[:].rearrange("p (k e) -> p k e", e=8)[:, :, 0],
                    op=mybir.AluOpType.add)
                hn = hpool.tile([P, 3 * D], BF16, tag="hn")
                for k in range(3):
                    nc.gpsimd.indirect_dma_start(
                        out=hn[:, k * D:(k + 1) * D], out_offset=None,
                        in_=hfull,
                        in_offset=bass.IndirectOffsetOnAxis(
                            ap=col3[:, k:k + 1], axis=0))
                hns[t] = hn
                # weight numerators: u = word^(1/64) == exp(v) * (1 +- 1.1%)
                nc.scalar.copy(out=uall[:, t * 3:(t + 1) * 3], in_=m8s[t][:, 0:3])

            def diff_stage(t, eng=None):
                """neighbor diffs (GpSimd bf16; DVE for the tail tile) +
                Square-accumulate (ACT)."""
                dif = hpool.tile([P, 3 * D], BF16, tag="dif")
                hb = hst[:, t * D:(t + 1) * D].unsqueeze(1).to_broadcast([P, 3, D])
                (eng or nc.gpsimd).tensor_tensor(
                    out=dif[:].rearrange("p (k d) -> p k d", k=3),
                    in0=hb, in1=hns[t][:].rearrange("p (k d) -> p k d", k=3),
                    op=mybir.AluOpType.subtract)
                for k in range(3):
                    nc.scalar.activation(
                        out=dif[:, k * D:(k + 1) * D],
                        in_=dif[:, k * D:(k + 1) * D],
                        func=mybir.ActivationFunctionType.Square,
                        accum_out=nrm2all[:, t * 3 + k:t * 3 + k + 1])

            # ---- interleaved schedule: keep the DVE fed with MAX8s while
            # gather chains for earlier tiles run on GpSimd/DMA. The bulk
            # tensor lands early, so the whole mse chain runs mid-kernel.
            sq = msep.tile([P, MSE_FD], BF16, tag="sq")
            find_stage(0)
            find_stage(1)
            rescan_stage(0)
            nc.scalar.activation(out=sq[:], in_=ht[:],
                                 func=mybir.ActivationFunctionType.Square,
                                 accum_out=res_t[:, 2:3])
            find_stage(2)
            rescan_stage(1)
            diff_stage(0)
            # mse residual chain: resid = ((x - h) + c) * m, in place
            nc.vector.tensor_tensor(out=xt[:], in0=xt[:], in1=ht[:],
                                    op=mybir.AluOpType.subtract)
            nc.scalar.activation(out=sq[:], in_=ct[:],
                                 func=mybir.ActivationFunctionType.Square,
                                 accum_out=res_t[:, 3:4])
            find_stage(3)
            rescan_stage(2)
            diff_stage(1)
            nc.vector.tensor_tensor(out=xt[:], in0=xt[:], in1=ct[:],
                                    op=mybir.AluOpType.add)
            nc.vector.tensor_tensor(out=xt[:], in0=xt[:], in1=mt[:],
                                    op=mybir.AluOpType.mult)
            rescan_stage(3)
            diff_stage(2)
            nc.scalar.activation(out=sq[:], in_=xt[:],
                                 func=mybir.ActivationFunctionType.Square,
                                 accum_out=res_t[:, 1:2])
            diff_stage(3, eng=nc.vector)

            # ---- phase B: softmax weights via six chained sqrts, then
            # weighted neighbor-norm dot, all on [P, 12].
            for _ in range(6):
                nc.scalar.sqrt(out=uall[:], in_=uall[:])
            nrmall = acc.tile([P, NT * 3], F32, tag="nrmall")
            nc.scalar.sqrt(out=nrmall[:], in_=nrm2all[:])
            s1 = acc.tile([P, NT], F32, tag="s1")
            nc.vector.tensor_reduce(
                out=s1[:], in_=uall[:].rearrange("p (t k) -> p t k", k=3),
                axis=mybir.AxisListType.X, op=mybir.AluOpType.add)
            r1 = acc.tile([P, NT], F32, tag="r1")
            nc.vector.reciprocal(out=r1[:], in_=s1[:])
            en = acc.tile([P, NT * 3], F32, tag="en")
            nc.vector.tensor_tensor(out=en[:], in0=uall[:], in1=nrmall[:],
                                    op=mybir.AluOpType.mult)
            dot = acc.tile([P, NT], F32, tag="dot")
            nc.vector.tensor_reduce(
                out=dot[:], in_=en[:].rearrange("p (t k) -> p t k", k=3),
                axis=mybir.AxisListType.X, op=mybir.AluOpType.add)
            simc = acc.tile([P, NT], F32, tag="simc")
            nc.vector.tensor_tensor(out=simc[:], in0=dot[:], in1=r1[:],
                                    op=mybir.AluOpType.mult)
            nc.vector.tensor_reduce(
                out=res_t[:, 0:1], in_=simc[:], axis=mybir.AxisListType.X,
                op=mybir.AluOpType.add)

            nc.sync.dma_start(out=out, in_=res_t[:])

    nc.compile()
    return nc


def _get_program():
    global _compiled
    if _compiled is None:
        _compiled = _build_program()
    return _compiled


def _pack_scores(row_scores, mc):
    """Negate + gather score rows, quantize the value into the bf16
    exponent field and the 128-column block id into the mantissa."""
    neg = -row_scores[mc]                                   # [R, N] f32
    m = neg.max(axis=1, keepdims=True)
    q = np.clip(np.rint((neg - m + QR) * QS), 0.0, 254.0).astype(np.uint16)
    blk = (np.arange(NCOLP, dtype=np.uint16) >> 7)          # [NCOLP]
    words = np.empty((R, NCOLP), dtype=np.uint16)
    words[:, :N] = (q << 7) | blk[:N]
    words[:, N:] = blk[N:]                                  # pad: q=0 losers
    return words.view(ml_dtypes.bfloat16)


def _make_in_maps(X, H, C, M, row_scores, mc_rows):
    mc = np.asarray(mc_rows).astype(np.int64)
    scores_p = _pack_scores(np.ascontiguousarray(row_scores), mc)
    bf = ml_dtypes.bfloat16
    Hb = H.astype(bf)
    hsel_g = Hb[mc]                                         # [R, D]
    pbase = (np.arange(P, dtype=np.uint32)[:, None] * NBLK
             + np.arange(NT, dtype=np.uint32)[None, :] * (P * NBLK))
    pbase_bits = pbase.view(np.uint16).view(bf)             # [P, NT*2]
    in_maps = []
    for c in range(NCORES):
        sl = slice(c * RPC, (c + 1) * RPC)
        rs = slice(c * SLC, (c + 1) * SLC)
        bulk = np.empty((P, BULK_COLS), dtype=bf)
        bulk[:, BO_X:BO_X + MSE_FD] = X[rs].astype(bf).reshape(P, MSE_FD)
        bulk[:, BO_H:BO_H + MSE_FD] = H[rs].astype(bf).reshape(P, MSE_FD)
        bulk[:, BO_C:BO_C + MSE_FD] = C[rs].astype(bf).reshape(P, MSE_FD)
        bulk[:, BO_M:BO_M + MSE_FD] = M[rs].astype(bf).reshape(P, MSE_FD)
        bulk[:, BO_HS:BO_HS + NT * D] = (
            hsel_g[sl].reshape(NT, P, D).transpose(1, 0, 2).reshape(P, NT * D))
        bulk[:, BO_PB:] = pbase_bits
        in_maps.append({
            "scores": np.ascontiguousarray(scores_p[sl]),
            "hfull": np.ascontiguousarray(Hb),
            "bulk": bulk,
        })
    return in_maps


def _finish(results):
    parts = np.stack([r["out"] for r in results]).astype(np.float64)  # [8,128,8]
    tot = parts.sum(axis=(0, 1))
    loss = tot[1] + tot[0] + 0.1 * np.sqrt(tot[3]) + 0.01 * np.sqrt(tot[2])
    return np.array(loss, dtype=np.float32)


def kernel(X, H, C, M, T, nM, row_scores, mc_rows, **_unused):
    X = np.asarray(X, dtype=np.float32)
    H = np.asarray(H, dtype=np.float32)
    C = np.asarray(C, dtype=np.float32)
    M = np.asarray(M, dtype=np.float32)
    row_scores = np.asarray(row_scores, dtype=np.float32)
    nc = _get_program()
    in_maps = _make_in_maps(X, H, C, M, row_scores, mc_rows)
    res = run_bass_kernel_spmd(nc, in_maps, list(range(NCORES)))
    return _finish(res.results)


def run_traced(X, H, C, M, T, nM, row_scores, mc_rows, **_unused):
    """Like kernel() but returns (loss, BassKernelResults) with trace."""
    nc = _get_program()
    in_maps = _make_in_maps(
        np.asarray(X, dtype=np.float32), np.asarray(H, dtype=np.float32),
        np.asarray(C, dtype=np.float32), np.asarray(M, dtype=np.float32),
        np.asarray(row_scores, dtype=np.float32), mc_rows)
    try:
        res = run_bass_kernel_spmd(nc, in_maps, list(range(NCORES)), trace=True)
    except ModuleNotFoundError:
        res = run_bass_kernel_spmd(nc, in_maps, list(range(NCORES)))
    return _finish(res.results), res


# revision 31
# speedup vs baseline: 1.0280x; 1.0188x over previous
"""Trainium2 Bass kernel for nn_ClusterLoss (topk_masking).

Strategy (8 NeuronCores, data-parallel over the 4096 selected rows):
  - Host shards mc_rows and the gathered row_scores rows across cores
    (512 rows/core). Each score is negated and quantized to a 16-bit
    word that is a valid positive bf16: the 8-bit exponent field holds
    the quantized relative value q (resolution ln2/64 so exp(v) ==
    2^(q/64)) and the 7-bit mantissa holds the 128-column block id.
    bf16 ordering == (value, block) lexicographic ordering, so MAX8 /
    max-reduce rank correctly and half the HBM bytes move vs f32.
  - Device, per 128-row tile: chunked MAX8 over the 10000 packed words
    -> top-3 (value, block) words. The 3 winner blocks are re-fetched
    with indirect DMA (3 x 128 words/row) and MAX8 + max_index recover
    the exact in-block position -> global column index. H rows (bf16)
    are gathered by that index; diff + Square-accumulate give the
    neighbor norms. Softmax weights come from the packed words
    directly: w^(1/64) (six chained Sqrts) == exp(v) modulo a <=1.1%
    mantissa factor that cancels in the normalization.
  - Masked-MSE residual and squared-norm partials run on a bf16 copy
    of a 1250-row slice of X/H/C/M, interleaved with the score stream.
  - Each core returns [128, 8] per-partition partial sums; host
    reduces and assembles the scalar loss.
"""

import sys

sys.path.insert(0, "/opt/trn_rl_repo")

import math

import ml_dtypes
import numpy as np

from concourse import bacc, bass, mybir, tile
from concourse.bass_utils import run_bass_kernel_spmd
from concourse.tile_rust import add_dep_helper

N, D, R = 10000, 256, 4096
NCORES = 8
RPC = R // NCORES          # score rows per core = 512
SLC = N // NCORES          # mse rows per core = 1250
P = 128
NT = RPC // P              # score row-tiles per core = 4
MSE_FD = SLC * D // P      # 2500
F32 = mybir.dt.float32
U32 = mybir.dt.uint32
U16 = mybir.dt.uint16
BF16 = mybir.dt.bfloat16

W = 128                    # columns per block (mantissa-encoded id)
NBLK = 80                  # blocks per padded row
NCOLP = NBLK * W           # padded row length = 10240
CHUNKS = [2560, 2560, 2560, 2320]   # covers the 10000 real columns

# bulk tensor column offsets: xs | hs | cs | ms | hsel | pbase bits
BO_X, BO_H, BO_C, BO_M = 0, MSE_FD, 2 * MSE_FD, 3 * MSE_FD
BO_HS = 4 * MSE_FD                  # 10000
BO_PB = BO_HS + NT * D              # 11024
BULK_COLS = BO_PB + NT * 2          # 11032 bf16 columns

QS = 64.0 / math.log(2.0)  # quant scale: exp(v) == 2**(q/64)
QR = 254.0 / QS            # representable value range below row max

_compiled = None


def _build_program():
    nc = bacc.Bacc("TRN2", target_bir_lowering=False, debug=False)

    scores = nc.dram_tensor("scores", [RPC, NCOLP], BF16, kind="ExternalInput").ap()
    hfull = nc.dram_tensor("hfull", [N, D], BF16, kind="ExternalInput").ap()
    bulk = nc.dram_tensor("bulk", [P, BULK_COLS], BF16, kind="ExternalInput").ap()
    out = nc.dram_tensor("out", [P, 8], F32, kind="ExternalOutput").ap()

    gview = scores.rearrange("r (b w) -> (r b) w", w=W)   # [RPC*NBLK, W]

    with tile.TileContext(nc) as tc:
        with (
            tc.tile_pool(name="sc", bufs=1) as sc_pool,
            tc.tile_pool(name="small", bufs=NT) as small,
            tc.tile_pool(name="hp", bufs=NT) as hpool,
            tc.tile_pool(name="acc", bufs=1) as acc,
            tc.tile_pool(name="mse", bufs=1) as msep,
        ):
            res_t = acc.tile([P, 8], F32, tag="res")
            nc.vector.memset(res_t[:], 0.0)
            nrm2all = acc.tile([P, NT * 3], F32, tag="n2all")
            uall = acc.tile([P, NT * 3], F32, tag="uall")

            # one bulk tile: xs | hs | cs | ms | hsel | pbase-bits
            bulkt = msep.tile([P, BULK_COLS], BF16, tag="bulkt")
            xt = bulkt[:, BO_X:BO_X + MSE_FD]
            ht = bulkt[:, BO_H:BO_H + MSE_FD]
            ct = bulkt[:, BO_C:BO_C + MSE_FD]
            mt = bulkt[:, BO_M:BO_M + MSE_FD]
            hst = bulkt[:, BO_HS:BO_HS + NT * D]
            baset = bulkt[:, BO_PB:BO_PB + NT * 2].bitcast(U32)   # [P, NT]

            # ---- DMA issue order (descriptor-count, not bytes, paces the
            # DGE): 2 full-width contiguous-row chunks per tile + 1 bulk.
            sc_tiles = [
                sc_pool.tile([P, NCOLP], BF16, tag=f"sc{t}", name=f"sct{t}")
                for t in range(NT)
            ]

            def queue_tile_dma(t, sizes):
                col = 0
                for w in sizes:
                    nc.sync.dma_start(
                        out=sc_tiles[t][:, col:col + w],
                        in_=scores[t * P:(t + 1) * P, col:col + w])
                    col += w

            # hsel+pbase early (needed by rescan/diff of tile 0); the big
            # mse block last — its consumers run in the tail when the DVE
            # is idle, so it must never delay the score stream.
            queue_tile_dma(0, CHUNKS)
            nc.sync.dma_start(out=bulkt[:, BO_HS:], in_=bulk[:, BO_HS:])
            queue_tile_dma(1, CHUNKS)
            queue_tile_dma(2, CHUNKS)
            nc.sync.dma_start(out=bulkt[:, 0:BO_HS], in_=bulk[:, 0:BO_HS])
            queue_tile_dma(3, CHUNKS)

            # ---- per-tile state
            m8s, bids, blk3s, top8s, poss, cols, hns = (
                [None] * NT, [None] * NT, [None] * NT, [None] * NT,
                [None] * NT, [None] * NT, [None] * NT)

            def find_stage(t):
                """chunk MAX8s + merge + block-id unpack (DVE), then
                GpSimd computes gather rows and fetches winner blocks."""
                m8h = small.tile([P, len(CHUNKS) * 8], BF16, tag="m8h")
                col = 0
                for h, w in enumerate(CHUNKS):
                    nc.vector.max(out=m8h[:, h * 8:(h + 1) * 8],
                                  in_=sc_tiles[t][:, col:col + w])
                    col += w
                m8 = small.tile([P, 8], BF16, tag="m8")
                nc.vector.max(out=m8[:], in_=m8h[:])
                bid16 = small.tile([P, 3], U16, tag="bid16")
                nc.vector.tensor_scalar(
                    out=bid16[:], in0=m8[:, 0:3].bitcast(U16), scalar1=W - 1,
                    scalar2=None, op0=mybir.AluOpType.bitwise_and)
                # bitVec ops can't change dtype; widen u16->u32 via add 0
                bid = small.tile([P, 3], U32, tag="bid")
                nc.vector.tensor_scalar(
                    out=bid[:], in0=bid16[:], scalar1=0,
                    scalar2=None, op0=mybir.AluOpType.add)
                rows = small.tile([P, 3], U32, tag="rows")
                nc.vector.tensor_tensor(
                    out=rows[:], in0=bid[:],
                    in1=baset[:, t:t + 1].to_broadcast([P, 3]),
                    op=mybir.AluOpType.add)
                blk3 = small.tile([P, 3 * W], BF16, tag="blk3")
                for k in range(3):
                    nc.gpsimd.indirect_dma_start(
                        out=blk3[:, k * W:(k + 1) * W], out_offset=None,
                        in_=gview,
                        in_offset=bass.IndirectOffsetOnAxis(
                            ap=rows[:, k:k + 1], axis=0))
                m8s[t], bids[t], blk3s[t] = m8, bid, blk3

            def rescan_stage(t):
                """Per-winner max_index in its own gathered block: the
                match position IS the in-block column offset, so
                col = bid*128 + pos with no segment-select math."""
                pos_all = small.tile([P, 24], U32, tag="pos_all")
                for k in range(3):
                    nc.vector.max_index(
                        out=pos_all[:, k * 8:(k + 1) * 8],
                        in_max=m8s[t][:, k:k + 1].to_broadcast([P, 8]),
                        in_values=blk3s[t][:, k * W:(k + 1) * W])
                colw = small.tile([P, 3], U32, tag="colw")
                nc.vector.tensor_scalar(
                    out=colw[:], in0=bids[t][:], scalar1=W, scalar2=None,
                    op0=mybir.AluOpType.mult)
                col3 = small.tile([P, 3], U32, tag="col3")
                nc.vector.tensor_tensor(
                    out=col3[:], in0=colw[:],
                    in1=pos_all[:].rearrange("p (k e) -> p k e", e=8)[:, :, 0],
                    op=mybir.AluOpType.add)
                hn = hpool.tile([P, 3 * D], BF16, tag="hn")
                for k in range(3):
                    nc.gpsimd.indirect_dma_start(
                        out=hn[:, k * D:(k + 1) * D], out_offset=None,
                        in_=hfull,
                        in_offset=bass.IndirectOffsetOnAxis(
                            ap=col3[:, k:k + 1], axis=0))
                hns[t] = hn
                # weight numerators: u = word^(1/64) == exp(v) * (1 +- 1.1%)
                nc.scalar.copy(out=uall[:, t * 3:(t + 1) * 3], in_=m8s[t][:, 0:3])

            def diff_stage(t, eng=None):
                """neighbor diffs (GpSimd bf16; DVE for the tail tile) +
                Square-accumulate (ACT)."""
                dif = hpool.tile([P, 3 * D], BF16, tag="dif")
                hb = hst[:, t * D:(t + 1) * D].unsqueeze(1).to_broadcast([P, 3, D])
                (eng or nc.gpsimd).tensor_tensor(
                    out=dif[:].rearrange("p (k d) -> p k d", k=3),
                    in0=hb, in1=hns[t][:].rearrange("p (k d) -> p k d", k=3),
                    op=mybir.AluOpType.subtract)
                for k in range(3):
                    nc.scalar.activation(
                        out=dif[:, k * D:(k + 1) * D],
                        in_=dif[:, k * D:(k + 1) * D],
                        func=mybir.ActivationFunctionType.Square,
                        accum_out=nrm2all[:, t * 3 + k:t * 3 + k + 1])

            # ---- interleaved schedule: keep the DVE fed with MAX8s while
            # gather chains for earlier tiles run on GpSimd/DMA. The bulk
            # tensor lands early, so the whole mse chain runs mid-kernel.
            sq = msep.tile([P, MSE_FD], BF16, tag="sq")
            find_stage(0)
            find_stage(1)
            rescan_stage(0)
            nc.scalar.activation(out=sq[:], in_=ht[:],
                                 func=mybir.ActivationFunctionType.Square,
                                 accum_out=res_t[:, 2:3])
            find_stage(2)
            rescan_stage(1)
            diff_stage(0)
            # mse residual chain: resid = ((x - h) + c) * m, in place
            nc.vector.tensor_tensor(out=xt[:], in0=xt[:], in1=ht[:],
                                    op=mybir.AluOpType.subtract)
            nc.scalar.activation(out=sq[:], in_=ct[:],
                                 func=mybir.ActivationFunctionType.Square,
                                 accum_out=res_t[:, 3:4])
            find_stage(3)
            rescan_stage(2)
            diff_stage(1)
            nc.vector.tensor_tensor(out=xt[:], in0=xt[:], in1=ct[:],
                                    op=mybir.AluOpType.add)
            nc.vector.tensor_tensor(out=xt[:], in0=xt[:], in1=mt[:],
                                    op=mybir.AluOpType.mult)
            rescan_stage(3)
            diff_stage(2)
            nc.scalar.activation(out=sq[:], in_=xt[:],
                                 func=mybir.ActivationFunctionType.Square,
                                 accum_out=res_t[:, 1:2])
            diff_stage(3, eng=nc.vector)

            # ---- phase B: softmax weights via six chained sqrts, then
            # weighted neighbor-norm dot, all on [P, 12].
            for _ in range(6):
                nc.scalar.sqrt(out=uall[:], in_=uall[:])
            nrmall = acc.tile([P, NT * 3], F32, tag="nrmall")
            nc.scalar.sqrt(out=nrmall[:], in_=nrm2all[:])
            s1 = acc.tile([P, NT], F32, tag="s1")
            nc.vector.tensor_reduce(
                out=s1[:], in_=uall[:].rearrange("p (t k) -> p t k", k=3),
                axis=mybir.AxisListType.X, op=mybir.AluOpType.add)
            r1 = acc.tile([P, NT], F32, tag="r1")
            nc.vector.reciprocal(out=r1[:], in_=s1[:])
            en = acc.tile([P, NT * 3], F32, tag="en")
            nc.vector.tensor_tensor(out=en[:], in0=uall[:], in1=nrmall[:],
                                    op=mybir.AluOpType.mult)
            dot = acc.tile([P, NT], F32, tag="dot")
            nc.vector.tensor_reduce(
                out=dot[:], in_=en[:].rearrange("p (t k) -> p t k", k=3),
                axis=mybir.AxisListType.X, op=mybir.AluOpType.add)
            simc = acc.tile([P, NT], F32, tag="simc")
            nc.vector.tensor_tensor(out=simc[:], in0=dot[:], in1=r1[:],
                                    op=mybir.AluOpType.mult)
            nc.vector.tensor_reduce(
                out=res_t[:, 0:1], in_=simc[:], axis=mybir.AxisListType.X,
                op=mybir.AluOpType.add)

            nc.sync.dma_start(out=out, in_=res_t[:])

    nc.compile()
    return nc


def _get_program():
    global _compiled
    if _compiled is None:
        _compiled = _build_program()
    return _compiled


def _pack_scores(row_scores, mc):
    """Negate + gather score rows, quantize the value into the bf16
    exponent field and the 128-column block id into the mantissa."""
    neg = -row_scores[mc]                                   # [R, N] f32
    m = neg.max(axis=1, keepdims=True)
    q = np.clip(np.rint((neg - m + QR) * QS), 0.0, 254.0).astype(np.uint16)
    blk = (np.arange(NCOLP, dtype=np.uint16) >> 7)          # [NCOLP]
    words = np.empty((R, NCOLP), dtype=np.uint16)
    words[:, :N] = (q << 7) | blk[:N]
    words[:, N:] = blk[N:]                                  # pad: q=0 losers
    return words.view(ml_dtypes.bfloat16)


def _make_in_maps(X, H, C, M, row_scores, mc_rows):
    mc = np.asarray(mc_rows).astype(np.int64)
    scores_p = _pack_scores(np.ascontiguousarray(row_scores), mc)
    bf = ml_dtypes.bfloat16
    Hb = H.astype(bf)
    hsel_g = Hb[mc]                                         # [R, D]
    pbase = (np.arange(P, dtype=np.uint32)[:, None] * NBLK
             + np.arange(NT, dtype=np.uint32)[None, :] * (P * NBLK))
    pbase_bits = pbase.view(np.uint16).view(bf)             # [P, NT*2]
    in_maps = []
    for c in range(NCORES):
        sl = slice(c * RPC, (c + 1) * RPC)
        rs = slice(c * SLC, (c + 1) * SLC)
        bulk = np.empty((P, BULK_COLS), dtype=bf)
        bulk[:, BO_X:BO_X + MSE_FD] = X[rs].astype(bf).reshape(P, MSE_FD)
        bulk[:, BO_H:BO_H + MSE_FD] = H[rs].astype(bf).reshape(P, MSE_FD)
        bulk[:, BO_C:BO_C + MSE_FD] = C[rs].astype(bf).reshape(P, MSE_FD)
        bulk[:, BO_M:BO_M + MSE_FD] = M[rs].astype(bf).reshape(P, MSE_FD)
        bulk[:, BO_HS:BO_HS + NT * D] = (
            hsel_g[sl].reshape(NT, P, D).transpose(1, 0, 2).reshape(P, NT * D))
        bulk[:, BO_PB:] = pbase_bits
        in_maps.append({
            "scores": np.ascontiguousarray(scores_p[sl]),
            "hfull": np.ascontiguousarray(Hb),
            "bulk": bulk,
        })
    return in_maps


def _finish(results):
    parts = np.stack([r["out"] for r in results]).astype(np.float64)  # [8,128,8]
    tot = parts.sum(axis=(0, 1))
    loss = tot[1] + tot[0] + 0.1 * np.sqrt(tot[3]) + 0.01 * np.sqrt(tot[2])
    return np.array(loss, dtype=np.float32)


def kernel(X, H, C, M, T, nM, row_scores, mc_rows, **_unused):
    X = np.asarray(X, dtype=np.float32)
    H = np.asarray(H, dtype=np.float32)
    C = np.asarray(C, dtype=np.float32)
    M = np.asarray(M, dtype=np.float32)
    row_scores = np.asarray(row_scores, dtype=np.float32)
    nc = _get_program()
    in_maps = _make_in_maps(X, H, C, M, row_scores, mc_rows)
    res = run_bass_kernel_spmd(nc, in_maps, list(range(NCORES)))
    return _finish(res.results)


def run_traced(X, H, C, M, T, nM, row_scores, mc_rows, **_unused):
    """Like kernel() but returns (loss, BassKernelResults) with trace."""
    nc = _get_program()
    in_maps = _make_in_maps(
        np.asarray(X, dtype=np.float32), np.asarray(H, dtype=np.float32),
        np.asarray(C, dtype=np.float32), np.asarray(M, dtype=np.float32),
        np.asarray(row_scores, dtype=np.float32), mc_rows)
    try:
        res = run_bass_kernel_spmd(nc, in_maps, list(range(NCORES)), trace=True)
    except ModuleNotFoundError:
        res = run_bass_kernel_spmd(nc, in_maps, list(range(NCORES)))
    return _finish(res.results), res
